# revision 1
# baseline (speedup 1.0000x reference)
"""Category-specific 2-layer MLP (MoE-style routing), expert-parallel on 8 NeuronCores.

Math (per sample b with category c = cat_ids[b]):
    h   = relu(x_flat[b] @ W1[c] + b1[c])      x_flat: [32, 4096], W1: [8, 4096, 1024]
    out = h @ W2[c] + b2[c]                    W2: [8, 1024, 512]

Sharding: expert-parallel. Core k holds ONLY category k's weights (16 MB W1 + 2 MB W2)
and computes the full dense MLP for all 32 samples; the host then gathers row b from
core cat_ids[b]. Per-core HBM traffic is ~18.6 MB (the minimum possible when all 8
categories are in use), vs 144 MB for weight replication.

Kernel layout per core (fp32 matmuls stream the MOVING operand at 4 cycles/row, so
keep the moving dim small: stream x^T / h^T at N=32, keep the big weights stationary):
  layer 1: hT[u] [128, 32] (u = 0..7 mid-tiles, one PSUM bank each) accumulated over
           32 K-tiles: lhsT (stationary) = W1[128t:128t+128, 128u:128u+128],
           rhs (moving) = x^T tile [128, 32]. Produces h already transposed for
           layer 2 — no on-chip transpose stage at all.
  bias+relu: ONE DVE scalar_tensor_tensor per mid-tile:
           ht_sb = max(hT_psum + b1T[:, u], 0)   (b1 transposed is per-PARTITION).
  layer 2: transposed too: oT[v] [128, 32] (v = 0..3) over 8 K-tiles:
           lhsT = W2[128u:128u+128, 128v:128v+128], rhs = hT[u] [128, 32];
           evict fuses the b2 add. Output leaves the chip as out^T [512, 32];
           the host gather undoes the transpose for free.
  W1 streams as 7 uneven DMAs ([8,8,8,4,2,1,1] K-tiles) — big slabs amortize
  per-DMA latency, the tiny last slab shortens the post-stream PE tail.

Toolchain constraint: this walrus build allows at most ONE sync-wait command per
instruction. The program is structured so every instruction acquires at most one
new semaphore:
  - every W1/W2 slab lives in its own SBUF tile (no slot reuse -> DMAs carry no waits);
  - the xt DMA is placed 8 positions before the first W1 slab DMA, so both land on the
    same HWDGE queue and one cumulative wait covers both;
  - a leading DVE "touch" of the bias tile acquires its queue semaphore before the
    fused bias ops (which then wait only on PE);
  - the kernel-tail drain is split into single-wait drains (_patch_tail_drain).
Verified by _assert_wait_budget at build time.
"""

import numpy as np

import concourse.bass as bass
import concourse.mybir as mybir
import concourse.tile_sem_assignment as _tsa
from concourse import tile
from concourse.bass_utils import run_bass_kernel_spmd

NUM_CAT = 8
B = 32
IN_DIM = 4096   # 16 * 256
MID = 1024
OUT = 512       # 16 * 32
P = 128
KT1 = IN_DIM // P    # 32 k-tiles for layer 1
KT2 = MID // P       # 8 mid-tiles (layer-1 out / layer-2 contraction)
NT = OUT // P        # 4 out-tiles
SLAB_SIZES = (8, 8, 8, 4, 2, 1, 1)  # k-tiles per W1 DMA; sum == KT1
F32 = mybir.dt.float32

HWDGE_QUEUES = 4


class _PatchHwdgeQueues:
    """Pin Tile's HWDGE round-robin to n queues during scheduling."""

    def __init__(self, n: int):
        self.n = n

    def __enter__(self):
        self._saved = _tsa.NUM_HWDGE_SEMS
        _tsa.NUM_HWDGE_SEMS = self.n
        return self

    def __exit__(self, *exc):
        _tsa.NUM_HWDGE_SEMS = self._saved
        return False


def _patch_tail_drain():
    """Split Tile's kernel-tail drain (one wait per live proc) into a chain of
    single-wait drains: this walrus build caps sync-wait commands per instruction
    and rejects the stock multi-wait drain."""
    if getattr(tile.TileContext, "_tail_drain_patched", False):
        return
    from concourse.vector_clock import ScopedClock, VectorClock

    def _drain_and_barrier(self, tick_clock, wait_clock):
        gc = tick_clock.global_clock
        n = len(gc)
        for p in range(n):
            if gc[p] <= 0:
                continue
            sub = [0] * n
            sub[p] = gc[p]
            d = self.nc.sync.drain()
            wait_clock.add_sem_waits(d.ins, ScopedClock({None: VectorClock(sub)}))
        self.nc.all_engine_barrier()
        assert self.sems is not None
        popped = self.nc._tile_sem_poison_stack.pop()
        assert popped is self._sem_poison
        self.nc.clear_and_free_semaphores(list(self.sems.allocated().values()))
        self.nc.all_engine_barrier()

    tile.TileContext._drain_and_barrier = _drain_and_barrier
    tile.TileContext._tail_drain_patched = True


_patch_tail_drain()


def _build_nc() -> bass.Bass:
    nc = bass.Bass()

    # xt[p, t, b] = x_flat[b, t*128 + p]: K-major layout so each DMA partition line
    # is one contiguous 4 KB segment.
    xt = nc.dram_tensor("xt", [P, KT1, B], F32, kind="ExternalInput")
    w1 = nc.dram_tensor("w1", [IN_DIM, MID], F32, kind="ExternalInput")
    w2 = nc.dram_tensor("w2", [MID, OUT], F32, kind="ExternalInput")
    # biast[p, 0:8] = b1[128u + p]; [p, 8:12] = b2[128v + p]; [p, 12] = 0.
    biast = nc.dram_tensor("biast", [P, KT2 + NT + 1], F32, kind="ExternalInput")
    out = nc.dram_tensor("out", [OUT, B], F32, kind="ExternalOutput")  # transposed

    with _PatchHwdgeQueues(HWDGE_QUEUES), tile.TileContext(nc) as tc:
        with (
            tc.tile_pool(name="const", bufs=1) as const,
            tc.tile_pool(name="w1p", bufs=1) as w1p,
            tc.tile_pool(name="w2p", bufs=1) as w2p,
            tc.tile_pool(name="work", bufs=1) as work,
            tc.tile_pool(name="psum", bufs=1, space="PSUM") as psum,
        ):
            # DMA issue order fixes HWDGE queue assignment (round-robin mod 4):
            # 0:xt 1:biast 2:w2a 3:w2b 4+:w1 slabs. xt (pos 0) and w1 slab 0
            # (pos 4) share a queue -> one cumulative wait covers both for the
            # first matmul. Each queue later carries a W1 slab, so every small
            # input is covered by the slab waits PE already performs.
            xt_sb = const.tile([P, KT1, B], F32)
            nc.sync.dma_start(xt_sb[:], xt[:])
            biast_sb = const.tile([P, KT2 + NT + 1], F32)
            nc.sync.dma_start(biast_sb[:], biast[:])

            # W2 in two 1 MB DMAs; w2_sbs[h][:, f, :] is K-tile 4h+f.
            w2_sbs = []
            for h in range(2):
                w2_sb = w2p.tile([P, KT2 // 2, OUT], F32, tag=f"w2_{h}", name=f"w2sb{h}")
                nc.sync.dma_start(
                    w2_sb[:],
                    w2[P * (KT2 // 2) * h : P * (KT2 // 2) * (h + 1), :].rearrange(
                        "(f p) n -> p f n", p=P
                    ),
                )
                w2_sbs.append(w2_sb)

            # W1 as 8 uneven DMAs; w1_sbs[s][:, f, :] is K-tile (slab_start[s] + f).
            w1_sbs = []
            row = 0
            slab_of_ktile = []
            for s, sz in enumerate(SLAB_SIZES):
                w1_sb = w1p.tile([P, sz, MID], F32, tag=f"w1_{s}", name=f"w1sb{s}")
                nc.sync.dma_start(
                    w1_sb[:],
                    w1[row : row + P * sz, :].rearrange("(f p) n -> p f n", p=P),
                )
                w1_sbs.append(w1_sb)
                slab_of_ktile += [(s, f) for f in range(sz)]
                row += P * sz

            # ---- layer 1: hT[u][128, 32] = (x @ W1)^T mid-tiles, 8 PSUM banks ----
            ht_ps = [
                psum.tile([P, B], F32, tag=f"hT_{u}", name=f"htps{u}")
                for u in range(KT2)
            ]
            for t in range(KT1):
                s, f = slab_of_ktile[t]
                for u in range(KT2):
                    nc.tensor.matmul(
                        ht_ps[u][:],
                        w1_sbs[s][:, f, P * u : P * (u + 1)],
                        xt_sb[:, t, :],
                        start=(t == 0),
                        stop=(t == KT1 - 1),
                    )

            # DVE touch: acquire the biast queue semaphore ahead of the fused
            # bias ops so they only ever wait on PE.
            touch_sb = work.tile([P, 1], F32)
            nc.vector.tensor_copy(touch_sb[:], biast_sb[:, 12:13])

            zero_bc = biast_sb[:, 12:13].to_broadcast((P, B))

            # ---- fused bias+relu evict: ht_sb[:,u,:] = max(hT[u] + b1T[:,u], 0) ----
            ht_sb = work.tile([P, KT2, B], F32)
            for u in range(KT2):
                nc.vector.scalar_tensor_tensor(
                    ht_sb[:, u, :],
                    ht_ps[u][:],
                    biast_sb[:, u : u + 1],
                    zero_bc,
                    mybir.AluOpType.add,
                    mybir.AluOpType.max,
                )

            # ---- layer 2 (transposed): oT[v][128, 32] over 8 K-tiles ----
            # oT psum tiles reuse hT_0..3 slots (released once their evict ran).
            ot_ps = [
                psum.tile([P, B], F32, tag=f"hT_{v}", name=f"otps{v}")
                for v in range(NT)
            ]
            for u in range(KT2):
                for v in range(NT):
                    nc.tensor.matmul(
                        ot_ps[v][:],
                        w2_sbs[u // 4][:, u % 4, P * v : P * (v + 1)],
                        ht_sb[:, u, :],
                        start=(u == 0),
                        stop=(u == KT2 - 1),
                    )

            # ---- fused bias evict: ot_sb[:,v,:] = oT[v] + b2T[:,v] ----
            ot_sb = work.tile([P, NT, B], F32)
            for v in range(NT):
                nc.vector.scalar_tensor_tensor(
                    ot_sb[:, v, :],
                    ot_ps[v][:],
                    biast_sb[:, KT2 + v : KT2 + v + 1],
                    zero_bc,
                    mybir.AluOpType.add,
                    mybir.AluOpType.add,
                )
            # SWDGE path: a fresh DMA proc, so the store carries only the DVE wait
            # (an HWDGE queue would add a self-queue FIFO wait -> 2 waits).
            nc.gpsimd.dma_start(out.rearrange("(v p) b -> p v b", p=P), ot_sb[:])

    _assert_wait_budget(nc)
    return nc


def _assert_wait_budget(nc: bass.Bass, max_waits: int = 1):
    """This walrus build rejects instructions with >1 sync wait; fail fast."""
    bad = []
    for blk in nc.m.functions[0].blocks:
        for inst in blk.instructions:
            if type(inst).__name__ not in (
                "InstMatmult",
                "InstDMACopy",
                "InstDrain",
                "InstTensorCopy",
                "InstTensorScalarPtr",
            ):
                continue
            si = inst.sync_info
            nw = len(si.on_wait) if si is not None else 0
            if nw > max_waits:
                bad.append(
                    (
                        inst.name,
                        type(inst).__name__,
                        [(w.ant_name, w.wait_value) for w in si.on_wait],
                    )
                )
    if bad:
        raise RuntimeError(f"instructions with >{max_waits} sync waits: {bad}")


_NC_CACHE: bass.Bass | None = None


def _get_nc() -> bass.Bass:
    global _NC_CACHE
    if _NC_CACHE is None:
        _NC_CACHE = _build_nc()
    return _NC_CACHE


def _make_in_maps(x, W1, b1, W2, b2):
    x_flat = np.ascontiguousarray(np.asarray(x, dtype=np.float32)).reshape(B, IN_DIM)
    # xt[p, t, b] = x_flat[b, t*128 + p]
    xt = np.ascontiguousarray(x_flat.reshape(B, KT1, P).transpose(2, 1, 0))
    W1 = np.ascontiguousarray(np.asarray(W1, dtype=np.float32))
    W2 = np.ascontiguousarray(np.asarray(W2, dtype=np.float32))
    b1 = np.asarray(b1, dtype=np.float32)
    b2 = np.asarray(b2, dtype=np.float32)
    biast = np.zeros((NUM_CAT, P, KT2 + NT + 1), dtype=np.float32)
    biast[:, :, :KT2] = b1.reshape(NUM_CAT, KT2, P).transpose(0, 2, 1)
    biast[:, :, KT2 : KT2 + NT] = b2.reshape(NUM_CAT, NT, P).transpose(0, 2, 1)
    return [
        {
            "xt": xt,
            "w1": W1[k],
            "w2": W2[k],
            "biast": biast[k],
        }
        for k in range(NUM_CAT)
    ]


def kernel(x, W1, b1, W2, b2, cat_ids) -> np.ndarray:
    nc = _get_nc()
    in_maps = _make_in_maps(x, W1, b1, W2, b2)
    res = run_bass_kernel_spmd(nc, in_maps, list(range(NUM_CAT))).results
    per_cat = np.stack([np.asarray(res[k]["out"]) for k in range(NUM_CAT)])  # [8, OUT, B]
    cat = np.asarray(cat_ids).astype(np.int64).reshape(B)
    sel = per_cat[cat, :, np.arange(B)]  # [B, OUT] (gather undoes the transpose)
    return np.ascontiguousarray(sel.reshape(B, 16, 32).astype(np.float32))



# revision 3
# speedup vs baseline: 2.4885x; 2.4885x over previous
"""Category-specific 2-layer MLP (MoE-style routing), expert-parallel on 8 NeuronCores.

Math (per sample b with category c = cat_ids[b]):
    h   = relu(x_flat[b] @ W1[c] + b1[c])      x_flat: [32, 4096], W1: [8, 4096, 1024]
    out = h @ W2[c] + b2[c]                    W2: [8, 1024, 512]

Sharding: expert-parallel. Core k holds ONLY category k's weights and computes the
full dense MLP for all 32 samples; the host gathers row b from core cat_ids[b].

The kernel is HBM-bound, so the whole design minimizes DMA bytes and keeps the
DMA engines saturated end-to-end:

  * W1 is shipped as float8_e3m4 scaled by 64 (4 MB instead of 16 MB fp32).
    Power-of-two scaling is exact in fp: the host folds x64 into b1 and /64 into
    W2, so h' = 64*h flows through layer 2 with no extra on-chip ops. Measured
    end-to-end rel err 1.4e-2 (tolerance 2e-2); x/h/W2 stay bf16 (PE allows
    mixed fp8 x bf16 matmuls).
  * Everything is pre-laid out on the host in SBUF order so every DMA partition
    line is one contiguous 4-8 KB segment (full modeled DMA bandwidth; <512 B
    descriptors would halve it).
  * u-major schedule: W1 streams as 8 column-blocks (mid-tiles). Block u's
    32 k-tile matmuls, its relu evict and its layer-2 matmuls all overlap the
    DMA of blocks u+1... Only the last block's tiny tail (the final k-tile slab
    is 1 k-tile) plus the output path trails the last DMA byte.
  * Biases are added on the PE via K=1 matmuls (lhsT = bias row [1,128],
    rhs = ones [1,32]) inside the PSUM accumulation groups, so the DVE evicts
    are a single tensor_scalar (relu+bf16 cast) / tensor_copy per tile.

Layer 1 (transposed): hT[u] [128, 32] over 32 K-tiles: lhsT (stationary) =
W1 column-block tile [128, 128], rhs (moving) = x^T k-tile [128, 32]. Layer 2:
oT[v] slices of one PSUM tile [128, 4, 32] over 8 K-tiles: lhsT = W2 tile,
rhs = hT[u]. Output leaves as out[p, v, b] = out^T[128v+p, b]; host gather
undoes the transpose for free.

Toolchain constraint: this walrus build allows at most ONE sync-wait command per
instruction. Structure guaranteeing that:
  - every DMA lands in its own SBUF tile (no slot reuse -> DMAs carry no waits);
  - 4 HWDGE queues, issue order chosen so each PE matmul group acquires at most
    one new queue semaphore (xt shares q0 with W1 block 0; w2/bias queues are
    subsumed by later W1-block waits before their consumers run);
  - PSUM ping-pong reuse (ht_ps[u%2]) is safe without a wait because PE already
    synced to the evicting DVE via the layer-2 matmuls of block u-2;
  - the kernel-tail drain is split into single-wait drains (_patch_tail_drain).
Verified by _assert_wait_budget at build time.
"""

import numpy as np
import ml_dtypes

import concourse.bass as bass
import concourse.mybir as mybir
import concourse.tile_sem_assignment as _tsa
from concourse import tile
from concourse.bass_utils import run_bass_kernel_spmd

NUM_CAT = 8
B = 32
IN_DIM = 4096   # 16 * 256
MID = 1024
OUT = 512       # 16 * 32
P = 128
KT1 = IN_DIM // P    # 32 k-tiles for layer 1
KT2 = MID // P       # 8 mid-tiles (layer-1 out / layer-2 contraction)
NT = OUT // P        # 4 out-tiles
W1S = 64.0           # exact power-of-2 scale for fp8 W1
U7_SLABS = (16, 8, 4, 2, 1, 1)  # k-tiles per DMA for the last W1 block
F32 = mybir.dt.float32
BF16 = mybir.dt.bfloat16
F8 = mybir.dt.float8e3  # e3m4

HWDGE_QUEUES = 4


class _PatchHwdgeQueues:
    """Pin Tile's HWDGE round-robin to n queues during scheduling."""

    def __init__(self, n: int):
        self.n = n

    def __enter__(self):
        self._saved = _tsa.NUM_HWDGE_SEMS
        _tsa.NUM_HWDGE_SEMS = self.n
        return self

    def __exit__(self, *exc):
        _tsa.NUM_HWDGE_SEMS = self._saved
        return False


def _patch_tail_drain():
    """Split Tile's kernel-tail drain (one wait per live proc) into a chain of
    single-wait drains: this walrus build caps sync-wait commands per instruction
    and rejects the stock multi-wait drain."""
    if getattr(tile.TileContext, "_tail_drain_patched", False):
        return
    from concourse.vector_clock import ScopedClock, VectorClock

    def _drain_and_barrier(self, tick_clock, wait_clock):
        gc = tick_clock.global_clock
        n = len(gc)
        for p in range(n):
            if gc[p] <= 0:
                continue
            sub = [0] * n
            sub[p] = gc[p]
            d = self.nc.sync.drain()
            wait_clock.add_sem_waits(d.ins, ScopedClock({None: VectorClock(sub)}))
        self.nc.all_engine_barrier()
        assert self.sems is not None
        popped = self.nc._tile_sem_poison_stack.pop()
        assert popped is self._sem_poison
        self.nc.clear_and_free_semaphores(list(self.sems.allocated().values()))
        self.nc.all_engine_barrier()

    tile.TileContext._drain_and_barrier = _drain_and_barrier
    tile.TileContext._tail_drain_patched = True


_patch_tail_drain()


def _build_nc() -> bass.Bass:
    nc = bass.Bass()

    # All DRAM layouts are pre-transposed on the host so each DMA partition
    # line is one contiguous segment.
    # xt[p, t, b] = x_flat[b, 128t + p]
    xt = nc.dram_tensor("xt", [P, KT1, B], BF16, kind="ExternalInput")
    # w1t[u, p, t, c] = 64 * W1[128t + p, 128u + c]   (fp8 e3m4)
    w1t = nc.dram_tensor("w1t", [KT2, P, KT1, P], F8, kind="ExternalInput")
    # w2t[p, u, n] = W2[128u + p, n] / 64
    w2t = nc.dram_tensor("w2t", [P, KT2, OUT], BF16, kind="ExternalInput")
    # biasv[0, 0:1024] = 64*b1; [0, 1024:1536] = b2; [0, 1536:1568] = 1.0
    biasv = nc.dram_tensor("biasv", [1, MID + OUT + B], BF16, kind="ExternalInput")
    # out[p, v, b] = out^T[128v + p, b]
    out = nc.dram_tensor("out", [P, NT, B], F32, kind="ExternalOutput")

    n_u7 = len(U7_SLABS)
    u7_start = [sum(U7_SLABS[:i]) for i in range(n_u7)]

    with _PatchHwdgeQueues(HWDGE_QUEUES), tile.TileContext(nc) as tc:
        with (
            tc.tile_pool(name="const", bufs=1) as const,
            tc.tile_pool(name="w1p", bufs=1) as w1p,
            tc.tile_pool(name="work", bufs=1) as work,
            tc.tile_pool(name="psum", bufs=1, space="PSUM") as psum,
        ):
            # DMA issue order fixes HWDGE queue assignment (round-robin mod 4).
            # idx0 xt->q0, idx1 biasv->q1, idx2 w2a->q2, idx3 w2b->q3, then W1
            # block u lands on queue u%4 (idx 4+u): block0 shares q0 with xt so
            # the first matmul's single cumulative wait covers both; L1 block-u
            # matmuls cumulatively cover the w2/bias queues before layer 2 and
            # the bias matmuls consume them.
            xt_sb = const.tile([P, KT1, B], BF16)
            nc.sync.dma_start(xt_sb[:], xt[:])
            biasv_sb = const.tile([1, MID + OUT + B], BF16)
            nc.sync.dma_start(biasv_sb[:], biasv[:])
            # W2 in two halves; w2_sbs[h][:, f, :] holds mid-tile u = 4h + f.
            w2_sbs = []
            for h in range(2):
                w2_sb = const.tile([P, KT2 // 2, OUT], BF16, name=f"w2sb{h}")
                nc.sync.dma_start(w2_sb[:], w2t[:, 4 * h : 4 * (h + 1), :])
                w2_sbs.append(w2_sb)

            # W1 column-blocks: u = 0..6 one DMA each; u = 7 in uneven slabs so
            # only one k-tile of matmul work trails the final DMA byte.
            w1_sbs = []  # per u: list of (tile, k_start, n_k)
            for u in range(KT2 - 1):
                w1_sb = w1p.tile([P, KT1, P], F8, tag=f"w1_{u}", name=f"w1sb{u}")
                nc.sync.dma_start(w1_sb[:], w1t[u])
                w1_sbs.append([(w1_sb, 0, KT1)])
            slabs = []
            for s, sz in enumerate(U7_SLABS):
                k0 = u7_start[s]
                w1_sb = w1p.tile([P, sz, P], F8, tag=f"w1_7{s}", name=f"w1sb7{s}")
                nc.sync.dma_start(w1_sb[:], w1t[KT2 - 1, :, k0 : k0 + sz, :])
                slabs.append((w1_sb, k0, sz))
            w1_sbs.append(slabs)

            ones_ap = biasv_sb[0:1, MID + OUT : MID + OUT + B]

            ht_ps = [
                psum.tile([P, B], F32, tag=f"hT_{i}", name=f"htps{i}")
                for i in range(2)
            ]
            ot_ps = psum.tile([P, NT, B], F32, tag="oT", name="otps")
            ht_sb = work.tile([P, KT2, B], BF16)
            ot_sb = work.tile([P, NT, B], F32)

            def l1_block(u):
                ps = ht_ps[u % 2]
                for w1_sb, k0, nk in w1_sbs[u]:
                    for f in range(nk):
                        nc.tensor.matmul(
                            ps[:],
                            w1_sb[:, f, :],
                            xt_sb[:, k0 + f, :],
                            start=(k0 + f == 0),
                            stop=False,
                        )
                # bias via K=1 matmul: hT[u] += (64*b1)[128u:128(u+1)] ^T @ ones
                nc.tensor.matmul(
                    ps[:],
                    biasv_sb[0:1, P * u : P * (u + 1)],
                    ones_ap,
                    start=False,
                    stop=True,
                )

            def evict_block(u):
                # relu + bf16 cast; bias already accumulated in PSUM
                nc.vector.tensor_scalar(
                    ht_sb[:, u, :], ht_ps[u % 2][:], 0.0, None,
                    mybir.AluOpType.max,
                )

            def l2_block(u):
                # ot_ps is one PSUM bank = one zero region: a single
                # accumulation group for all 4 v slices. start on the very
                # first matmul zeroes the whole bank; stop on the very last.
                for v in range(NT):
                    nc.tensor.matmul(
                        ot_ps[:, v, :],
                        w2_sbs[u // 4][:, u % 4, P * v : P * (v + 1)],
                        ht_sb[:, u, :],
                        start=(u == 0 and v == 0),
                        stop=(u == KT2 - 1 and v == NT - 1),
                    )
                if u == 0:
                    # out bias via K=1 matmuls inside the open accumulation group
                    for v in range(NT):
                        nc.tensor.matmul(
                            ot_ps[:, v, :],
                            biasv_sb[0:1, MID + P * v : MID + P * (v + 1)],
                            ones_ap,
                            start=False,
                            stop=False,
                        )

            # Software pipeline: L1(u) runs while block u+1 streams; L2(u-1)
            # sits after L1(u) so the DVE evict never stalls the PE and the
            # PE->DVE sync it carries covers the ht_ps ping-pong reuse.
            l1_block(0)
            evict_block(0)
            l1_block(1)
            evict_block(1)
            l2_block(0)
            for u in range(2, KT2):
                l1_block(u)
                evict_block(u)
                l2_block(u - 1)
            l2_block(KT2 - 1)

            nc.vector.tensor_copy(ot_sb[:], ot_ps[:])
            # SWDGE path: a fresh DMA proc, so the store carries only the DVE
            # wait (an HWDGE queue would add a self-queue FIFO wait -> 2 waits).
            nc.gpsimd.dma_start(out[:], ot_sb[:])

    _assert_wait_budget(nc)
    return nc


def _assert_wait_budget(nc: bass.Bass, max_waits: int = 1):
    """This walrus build rejects instructions with >1 sync wait; fail fast."""
    bad = []
    for blk in nc.m.functions[0].blocks:
        for inst in blk.instructions:
            si = inst.sync_info
            nw = len(si.on_wait) if si is not None else 0
            if nw > max_waits:
                bad.append(
                    (
                        inst.name,
                        type(inst).__name__,
                        [(w.ant_name, w.wait_value) for w in si.on_wait],
                    )
                )
    if bad:
        raise RuntimeError(f"instructions with >{max_waits} sync waits: {bad}")


_NC_CACHE: bass.Bass | None = None


def _get_nc() -> bass.Bass:
    global _NC_CACHE
    if _NC_CACHE is None:
        _NC_CACHE = _build_nc()
    return _NC_CACHE


def _make_in_maps(x, W1, b1, W2, b2):
    bf16 = ml_dtypes.bfloat16
    f8 = ml_dtypes.float8_e3m4
    x_flat = np.asarray(x, dtype=np.float32).reshape(B, IN_DIM)
    # xt[p, t, b] = x_flat[b, 128t + p]
    xt = np.ascontiguousarray(
        x_flat.reshape(B, KT1, P).transpose(2, 1, 0).astype(bf16)
    )
    W1 = np.asarray(W1, dtype=np.float32)
    W2 = np.asarray(W2, dtype=np.float32)
    b1 = np.asarray(b1, dtype=np.float32)
    b2 = np.asarray(b2, dtype=np.float32)
    # w1t[k][u, p, t, c] = 64 * W1[k, 128t + p, 128u + c], e3m4
    w1t = np.ascontiguousarray(
        (W1 * W1S).reshape(NUM_CAT, KT1, P, KT2, P).transpose(0, 3, 2, 1, 4)
    ).astype(f8)
    # w2t[k][p, u, n] = W2[k, 128u + p, n] / 64
    w2t = np.ascontiguousarray(
        (W2 / W1S).reshape(NUM_CAT, KT2, P, OUT).transpose(0, 2, 1, 3).astype(bf16)
    )
    biasv = np.concatenate(
        [b1 * W1S, b2, np.ones((NUM_CAT, B), np.float32)], axis=1
    ).astype(bf16)[:, None, :]  # [8, 1, 1568]
    return [
        {
            "xt": xt,
            "w1t": np.ascontiguousarray(w1t[k]),
            "w2t": w2t[k],
            "biasv": biasv[k],
        }
        for k in range(NUM_CAT)
    ]


def kernel(x, W1, b1, W2, b2, cat_ids) -> np.ndarray:
    nc = _get_nc()
    in_maps = _make_in_maps(x, W1, b1, W2, b2)
    res = run_bass_kernel_spmd(nc, in_maps, list(range(NUM_CAT))).results
    per_cat = np.stack(
        [np.asarray(res[k]["out"], dtype=np.float32) for k in range(NUM_CAT)]
    )  # [8, P, NT, B]
    cat = np.asarray(cat_ids).astype(np.int64).reshape(B)
    sel = per_cat[cat, :, :, np.arange(B)]  # [B, P, NT]
    out = sel.transpose(0, 2, 1).reshape(B, OUT)  # o = 128v + p
    return np.ascontiguousarray(out.reshape(B, 16, 32).astype(np.float32))


# revision 24
# speedup vs baseline: 5.8228x; 2.3399x over previous
"""Category-specific 2-layer MLP (MoE-style routing), expert-parallel on 8 NeuronCores.

Math (per sample b with category c = cat_ids[b]):
    h   = relu(x_flat[b] @ W1[c] + b1[c])      x_flat: [32, 4096], W1: [8, 4096, 1024]
    out = h @ W2[c] + b2[c]                    W2: [8, 1024, 512]

Sharding: expert-parallel. Core k holds ONLY category k's weights and computes the
full dense MLP for all 32 samples; the host gathers row b from core cat_ids[b].

The kernel is DMA-bound. Two levers drive the design:

  * W1 ships as float8_e3m4 scaled by 64 (1 byte/elem). Power-of-two scaling is
    exact in fp: the host folds x64 into b1 and /64 into W2, so h' = 64*h flows
    through layer 2 with no extra on-chip ops. Measured end-to-end rel err
    1.4e-2 (tolerance 2e-2); x/h/W2 stay bf16 (PE allows mixed-dtype matmuls).
  * DMA issue cost serializes per ISSUING engine, so the stream is split across
    all three DMA-capable engines - SP + Activation (HWDGE) and Pool (SWDGE) -
    into three balanced chains that run concurrently.

Layout: everything is pre-transposed on the host so each DMA partition line is
one contiguous segment; the small bias vectors ride inside the xt tile so no
DMA pays the 500 ns descriptor-generation floor for them alone. W1 streams as
8 column-blocks (mid-tiles) in u-major order: block u's 32 k-tile matmuls, its
relu evict (DVE) and its layer-2 matmuls all overlap later blocks' DMAs. The
last block is split across the Act/Pool chains so little work trails the last
DMA byte. A PE warmup burst at t~0 rides out the tensor engine's 3 us p-state
ramp, so every real matmul runs at the full-speed clock; with that, the middle
of the kernel is PE-throughput-bound and the ends are DMA-latency-bound.

Layer 1 (transposed): hT[u] [128, 32] accumulated over 32 K-tiles: lhsT
(stationary) = W1 block tile [128, 128] fp8, rhs (moving) = x^T k-tile
[128, 32] bf16. Evict u: one DVE scalar_tensor_tensor = (hT + 64*b1T[:,u])
relu'd against a zero broadcast, cast to bf16. Layer 2: oT v-slices of one
PSUM bank over 8 K-tiles: lhsT = W2 tile bf16 (pre-divided by 64), rhs =
hT[u]; one accumulation group for the whole bank (start zeroes the bank; the
final evict adds the pre-broadcast b2 and writes fp32). Output leaves as
out[p, v*32+b] = out^T[128v+p, b]; the host gather undoes the transpose.

Toolchain constraint: this walrus build allows at most ONE sync-wait command
per instruction. Structure guaranteeing that:
  - every DMA lands in its own SBUF tile (no slot reuse -> DMAs carry no waits);
  - HWDGE queue assignment is round-robin over the combined SP+Act issue order;
    the interleave is chosen so every PE matmul group needs at most one new
    queue semaphore (xt shares q0 with W1 block 0; the w2 halves' queues are
    re-waited at higher values by later W1-block matmuls before layer 2 runs);
  - 4-deep PSUM ping-pong (ht_ps[u%4]): the start=True overwrite of block u is
    ordered after layer-2 of block u-4, whose wait on the DVE evict semaphore
    already covers the reuse - no second wait;
  - DVE "touches" of the xt and w2 tiles acquire those DMA-queue semaphores on
    the DVE clock ahead of the evicts; layer-2 matmuls and the bias evicts
    inherit that coverage transitively and wait only on one semaphore;
  - the kernel-tail drain is split into single-wait drains (_patch_tail_drain).
Verified by _assert_wait_budget at build time.
"""

import numpy as np
import ml_dtypes

import concourse.bass as bass
import concourse.mybir as mybir
import concourse.tile_sem_assignment as _tsa
from concourse import tile
from concourse.bass_utils import run_bass_kernel_spmd

NUM_CAT = 8
B = 32
IN_DIM = 4096   # 16 * 256
MID = 1024
OUT = 512       # 16 * 32
P = 128
KT1 = IN_DIM // P    # 32 k-tiles for layer 1
KT2 = MID // P       # 8 mid-tiles (layer-1 out / layer-2 contraction)
NT = OUT // P        # 4 out-tiles
W1S = 64.0           # exact power-of-2 scale for fp8 W1
# Last W1 block's k-tiles split across the Act and Pool chains.
U7_SLABS = ((0, 18), (18, 14))
N_WARMUP = 10       # PE p-state warmup matmuls (bridge until first W1 block lands)
XB = KT1 * B         # 1024: xt columns inside the packed xt+bias tile
# packed tile columns: [0:1024] xt, [1024:1032] 64*b1T, [1032] zero,
# [1033:1161] b2 broadcast
XCOLS = XB + KT2 + 1 + NT * B
F32 = mybir.dt.float32
BF16 = mybir.dt.bfloat16
F8 = mybir.dt.float8e3  # e3m4

HWDGE_QUEUES = 4


class _PatchHwdgeQueues:
    """Pin Tile's HWDGE round-robin to n queues during scheduling."""

    def __init__(self, n: int):
        self.n = n

    def __enter__(self):
        self._saved = _tsa.NUM_HWDGE_SEMS
        _tsa.NUM_HWDGE_SEMS = self.n
        return self

    def __exit__(self, *exc):
        _tsa.NUM_HWDGE_SEMS = self._saved
        return False


def _patch_tail_drain():
    """Split Tile's kernel-tail drain (one wait per live proc) into a chain of
    single-wait drains: this walrus build caps sync-wait commands per instruction
    and rejects the stock multi-wait drain."""
    if getattr(tile.TileContext, "_tail_drain_patched", False):
        return
    from concourse.vector_clock import ScopedClock, VectorClock

    def _drain_and_barrier(self, tick_clock, wait_clock):
        gc = tick_clock.global_clock
        n = len(gc)
        for p in range(n):
            if gc[p] <= 0:
                continue
            sub = [0] * n
            sub[p] = gc[p]
            d = self.nc.sync.drain()
            wait_clock.add_sem_waits(d.ins, ScopedClock({None: VectorClock(sub)}))
        self.nc.all_engine_barrier()
        assert self.sems is not None
        popped = self.nc._tile_sem_poison_stack.pop()
        assert popped is self._sem_poison
        self.nc.clear_and_free_semaphores(list(self.sems.allocated().values()))
        self.nc.all_engine_barrier()

    tile.TileContext._drain_and_barrier = _drain_and_barrier
    tile.TileContext._tail_drain_patched = True


_patch_tail_drain()


def _build_nc() -> bass.Bass:
    nc = bass.Bass()

    # All DRAM layouts are pre-transposed on the host so each DMA partition
    # line is one contiguous segment.
    # Packed xt+bias tile (bf16): [p, 32t+b] = x_flat[b, 128t+p] for col<1024;
    # [p, 1024+u] = 64*b1[128u+p]; [p, 1032] = 0; [p, 1033+32v+b] = b2[128v+p]
    xt = nc.dram_tensor("xt", [P, XCOLS], BF16, kind="ExternalInput")
    # w1t[u, p, t, c] = 64 * W1[128t + p, 128u + c]   (fp8 e3m4)
    w1t = nc.dram_tensor("w1t", [KT2, P, KT1, P], F8, kind="ExternalInput")
    # w2t[p, u, n] = W2[128u + p, n] / 64
    w2t = nc.dram_tensor("w2t", [P, KT2, OUT], BF16, kind="ExternalInput")
    # out[p, v*32 + b] = out^T[128v + p, b]
    out = nc.dram_tensor("out", [P, NT * B], F32, kind="ExternalOutput")

    with _PatchHwdgeQueues(HWDGE_QUEUES), tile.TileContext(nc) as tc:
        with (
            tc.tile_pool(name="const", bufs=1) as const,
            tc.tile_pool(name="w1p", bufs=1) as w1p,
            tc.tile_pool(name="work", bufs=1) as work,
            tc.tile_pool(name="psum", bufs=1, space="PSUM") as psum,
        ):
            # --- DMA plan: three concurrent chains -------------------------
            #  SP  (HWDGE): xt+bias, w1u0, w1u3, w1u6
            #  Act (HWDGE): w2a, w1u1, w2b, w1u7a(18kt)
            #  Pool(SWDGE): w1u2, w1u4, w1u5, w1u7b(14kt), out
            # HWDGE queue = global (SP+Act) issue index mod 4; the interleave
            # below pins: xt q0, w2a q1, u1 q2, w2b q3, u0 q0, u3 q1, u6 q2,
            # u7a q3.  Single-wait coverage: L1u0 waits q0>=2 (xt+u0); L1u3
            # q1>=2 covers w2a; L1u6 q2>=2 covers u1; L1u7a q3>=2 covers w2b.
            # Layer-2 blocks and the evicts get their w2/bias queue coverage
            # transitively through the DVE touches below.
            w1_sbs: dict = {}

            def w1_tile(u, k0, nk, name):
                t = w1p.tile([P, nk, P], F8, tag=f"w1_{name}", name=f"w1sb{name}")
                w1_sbs.setdefault(u, []).append((t, k0, nk))
                return t, w1t[u, :, k0 : k0 + nk, :]

            xt_sb = const.tile([P, XCOLS], BF16)
            nc.sync.dma_start(xt_sb[:], xt[:])                      # SP   q0
            w2_sbs = [
                const.tile([P, KT2 // 2, OUT], BF16, name=f"w2sb{h}")
                for h in range(2)
            ]
            nc.scalar.dma_start(w2_sbs[0][:], w2t[:, 0:4, :])       # Act  q1
            t, src = w1_tile(1, 0, KT1, "1")
            nc.scalar.dma_start(t[:], src)                          # Act  q2
            nc.scalar.dma_start(w2_sbs[1][:], w2t[:, 4:8, :])       # Act  q3
            t, src = w1_tile(0, 0, KT1, "0")
            nc.sync.dma_start(t[:], src)                            # SP   q0
            t, src = w1_tile(3, 0, KT1, "3")
            nc.sync.dma_start(t[:], src)                            # SP   q1
            t, src = w1_tile(6, 0, KT1, "6")
            nc.sync.dma_start(t[:], src)                            # SP   q2
            k0, nk = U7_SLABS[0]
            t, src = w1_tile(7, k0, nk, "7a")
            nc.scalar.dma_start(t[:], src)                          # Act  q3
            # Pool chain (SWDGE lanes, own semaphore space)
            for u in (2, 4, 5):
                t, src = w1_tile(u, 0, KT1, str(u))
                nc.gpsimd.dma_start(t[:], src)
            k0, nk = U7_SLABS[1]
            t, src = w1_tile(7, k0, nk, "7b")
            nc.gpsimd.dma_start(t[:], src)
            # u7 sub-slabs must be consumed in issue order per engine; sort by k0
            w1_sbs[7].sort(key=lambda e: e[1])

            ht_ps = [
                psum.tile([P, B], F32, tag=f"hT_{i}", name=f"htps{i}")
                for i in range(4)
            ]
            ot_ps = psum.tile([P, NT * B], F32, tag="oT", name="otps")
            warm_ps = psum.tile([1, OUT], F32, tag="warm", name="warmps")
            ht_sb = work.tile([P, KT2, B], BF16)
            ot_sb = work.tile([P, NT * B], F32)

            zero_bc = xt_sb[:, XB + KT2 : XB + KT2 + 1].to_broadcast((P, B))

            # PE warmup: the tensor engine ramps to full clock only after 3us
            # of sustained work. A dozen dummy matmuls on a zeroed tile keep it
            # busy from t~0 so every real matmul runs at the ramped rate.
            warm_sb = work.tile([1, OUT], BF16)
            nc.vector.memset(warm_sb[:], 0.0)
            for _ in range(N_WARMUP):
                nc.tensor.matmul(
                    warm_ps[:], warm_sb[0:1, 0:1], warm_sb[:], start=True, stop=True
                )

            def l1_block(u):
                ps = ht_ps[u % 4]
                for w1_sb, k0, nk in w1_sbs[u]:
                    for f in range(nk):
                        t = k0 + f
                        nc.tensor.matmul(
                            ps[:],
                            w1_sb[:, f, :],
                            xt_sb[:, B * t : B * (t + 1)],
                            start=(t == 0),
                            stop=(t == KT1 - 1),
                        )

            def evict_block(u):
                # ht = max(hT[u] + 64*b1T[:,u], 0), cast to bf16
                nc.vector.scalar_tensor_tensor(
                    ht_sb[:, u, :],
                    ht_ps[u % 4][:],
                    xt_sb[:, XB + u : XB + u + 1],
                    zero_bc,
                    mybir.AluOpType.add,
                    mybir.AluOpType.max,
                )

            def l2_block(u):
                # ot_ps is one PSUM bank = one zero region: a single
                # accumulation group for all 4 v slices. start on the very
                # first matmul zeroes the whole bank; stop on the very last.
                for v in range(NT):
                    nc.tensor.matmul(
                        ot_ps[:, B * v : B * (v + 1)],
                        w2_sbs[u // 4][:, u % 4, P * v : P * (v + 1)],
                        ht_sb[:, u, :],
                        start=(u == 0 and v == 0),
                        stop=(u == KT2 - 1 and v == NT - 1),
                    )

            # DVE touches: acquire the xt/w2a/w2b queue semaphores on the
            # DVE clock ahead of the evicts that layer 2 waits on. Every
            # layer-2 matmul waits on a DVE evict that postdates the touch of
            # the w2 half it reads, so the w2 queue coverage reaches the PE
            # transitively - layer-2 blocks then carry only the one DVE wait
            # and can be scheduled anywhere. The w2b touch sits after e3 (its
            # first consumer is l2(4), which waits e4) so the early evicts
            # are not queued behind w2b's later-arriving DMA.
            touch_sb = work.tile([P, 3], F32)
            nc.vector.tensor_copy(touch_sb[:, 0:1], xt_sb[:, XB : XB + 1])
            nc.vector.tensor_copy(touch_sb[:, 1:2], w2_sbs[0][:, 0, 0:1])

            # Software pipeline. L2(u) sits 4 L1 blocks behind so the DVE-evict
            # sync it carries covers the ht_ps[u%4] reuse by L1(u+4). All of
            # layer 2 except block 7 runs before l1(7), so only e7 + l2(7) +
            # the final evict trail the last W1 bytes.
            l1_block(0)
            evict_block(0)
            l1_block(1)
            evict_block(1)
            l1_block(2)
            evict_block(2)
            l1_block(3)
            evict_block(3)
            nc.vector.tensor_copy(touch_sb[:, 2:3], w2_sbs[1][:, 0, 0:1])
            l2_block(0)
            for u in range(4, KT2 - 1):
                l1_block(u)
                evict_block(u)
                l2_block(u - 3)
            for u in range(KT2 - 4, KT2 - 1):
                l2_block(u)
            l1_block(KT2 - 1)
            evict_block(KT2 - 1)
            l2_block(KT2 - 1)

            # Final evict fuses the b2 add: ot = (oT + 0) + b2bc
            nc.vector.scalar_tensor_tensor(
                ot_sb[:],
                ot_ps[:],
                0.0,
                xt_sb[:, XB + KT2 + 1 : XCOLS],
                mybir.AluOpType.add,
                mybir.AluOpType.add,
            )
            # SWDGE store: a fresh DMA proc, so it carries only the DVE wait
            # (an HWDGE queue would add a FIFO wait -> 2 waits, and measures
            # slower besides).
            nc.gpsimd.dma_start(out[:], ot_sb[:])

    _assert_wait_budget(nc)
    return nc


def _assert_wait_budget(nc: bass.Bass, max_waits: int = 1):
    """This walrus build rejects instructions with >1 sync wait; fail fast."""
    bad = []
    for blk in nc.m.functions[0].blocks:
        for inst in blk.instructions:
            si = inst.sync_info
            nw = len(si.on_wait) if si is not None else 0
            if nw > max_waits:
                bad.append(
                    (
                        inst.name,
                        type(inst).__name__,
                        [(w.ant_name, w.wait_value) for w in si.on_wait],
                    )
                )
    if bad:
        raise RuntimeError(f"instructions with >{max_waits} sync waits: {bad}")


_NC_CACHE: bass.Bass | None = None


def _get_nc() -> bass.Bass:
    global _NC_CACHE
    if _NC_CACHE is None:
        _NC_CACHE = _build_nc()
    return _NC_CACHE


def _make_in_maps(x, W1, b1, W2, b2):
    bf16 = ml_dtypes.bfloat16
    f8 = ml_dtypes.float8_e3m4
    x_flat = np.asarray(x, dtype=np.float32).reshape(B, IN_DIM)
    W1 = np.asarray(W1, dtype=np.float32)
    W2 = np.asarray(W2, dtype=np.float32)
    b1 = np.asarray(b1, dtype=np.float32)
    b2 = np.asarray(b2, dtype=np.float32)
    # Packed xt+bias tile, per category (xt part shared): [p, 32t+b] = x^T;
    # then 64*b1T, a zero column, and b2 broadcast along the batch dim.
    xtb = np.zeros((NUM_CAT, P, XCOLS), np.float32)
    xtb[:, :, :XB] = (
        x_flat.reshape(B, KT1, P).transpose(2, 1, 0).reshape(1, P, XB)
    )
    xtb[:, :, XB : XB + KT2] = (b1 * W1S).reshape(NUM_CAT, KT2, P).transpose(0, 2, 1)
    b2t = b2.reshape(NUM_CAT, NT, P).transpose(0, 2, 1)  # [k, p, v]
    xtb[:, :, XB + KT2 + 1 :] = np.repeat(b2t, B, axis=2)
    xtb = np.ascontiguousarray(xtb.astype(bf16))
    # w1t[k][u, p, t, c] = 64 * W1[k, 128t + p, 128u + c], e3m4
    w1t = np.ascontiguousarray(
        (W1 * W1S).reshape(NUM_CAT, KT1, P, KT2, P).transpose(0, 3, 2, 1, 4)
    ).astype(f8)
    # w2t[k][p, u, n] = W2[k, 128u + p, n] / 64
    w2t = np.ascontiguousarray(
        (W2 / W1S).reshape(NUM_CAT, KT2, P, OUT).transpose(0, 2, 1, 3).astype(bf16)
    )
    return [
        {
            "xt": xtb[k],
            "w1t": np.ascontiguousarray(w1t[k]),
            "w2t": w2t[k],
        }
        for k in range(NUM_CAT)
    ]


def kernel(x, W1, b1, W2, b2, cat_ids) -> np.ndarray:
    nc = _get_nc()
    in_maps = _make_in_maps(x, W1, b1, W2, b2)
    res = run_bass_kernel_spmd(nc, in_maps, list(range(NUM_CAT))).results
    per_cat = np.stack(
        [np.asarray(res[k]["out"], dtype=np.float32) for k in range(NUM_CAT)]
    )  # [8, P, NT*B]
    cat = np.asarray(cat_ids).astype(np.int64).reshape(B)
    pc = per_cat.reshape(NUM_CAT, P, NT, B)
    sel = pc[cat, :, :, np.arange(B)]  # [B, P, NT]
    out = sel.transpose(0, 2, 1).reshape(B, OUT)  # o = 128v + p
    return np.ascontiguousarray(out.reshape(B, 16, 32).astype(np.float32))


# revision 36
# speedup vs baseline: 7.0080x; 1.2036x over previous
"""Category-specific 2-layer MLP (MoE-style routing), expert-parallel on 8 NeuronCores.

Math (per sample b with category c = cat_ids[b]):
    h   = relu(x_flat[b] @ W1[c] + b1[c])      x_flat: [32, 4096], W1: [8, 4096, 1024]
    out = h @ W2[c] + b2[c]                    W2: [8, 1024, 512]

Sharding: expert-parallel. Core k holds ONLY category k's weights and computes the
full dense MLP for all 32 samples; the host gathers row b from core cat_ids[b].

The kernel is DMA-bound. Two levers drive the design:

  * W1 ships as float8_e3m4 scaled by 64 (1 byte/elem). Power-of-two scaling is
    exact in fp: the host folds x64 into b1 and /64 into W2, so h' = 64*h flows
    through layer 2 with no extra on-chip ops. Measured end-to-end rel err
    1.4e-2 (tolerance 2e-2); x/h/W2 stay bf16 (PE allows mixed-dtype matmuls).
  * DMA issue cost serializes per ISSUING engine, so the stream is split across
    all three DMA-capable engines - SP + Activation (HWDGE) and Pool (SWDGE) -
    into three balanced chains that run concurrently.

Layout: everything is pre-transposed on the host so each DMA partition line is
one contiguous segment; the small bias vectors ride inside the xt tile so no
DMA pays the 500 ns descriptor-generation floor for them alone. W1 streams as
8 column-blocks (mid-tiles) in u-major order: block u's 32 k-tile matmuls, its
relu evict (DVE) and its layer-2 matmuls all overlap later blocks' DMAs. The
last block is split across the Act/Pool chains so little work trails the last
DMA byte. A PE warmup burst at t~0 rides out the tensor engine's 3 us p-state
ramp, so every real matmul runs at the full-speed clock; with that, the middle
of the kernel is PE-throughput-bound and the ends are DMA-latency-bound.

Layer 1 (transposed): hT[u] [128, 32] accumulated over 32 K-tiles: lhsT
(stationary) = W1 block tile [128, 128] fp8, rhs (moving) = x^T k-tile
[128, 32] bf16. Evict u: one DVE scalar_tensor_tensor = (hT + 64*b1T[:,u])
relu'd against a zero broadcast, cast to bf16. Layer 2: oT v-slices of one
PSUM bank over 8 K-tiles: lhsT = W2 tile bf16 (pre-divided by 64), rhs =
hT[u]; one accumulation group for the whole bank (start zeroes the bank; the
final evict adds the pre-broadcast b2 and writes fp32). Output leaves as
out[p, v*32+b] = out^T[128v+p, b]; the host gather undoes the transpose.

Toolchain constraint: this walrus build allows at most ONE sync-wait command
per instruction. Structure guaranteeing that:
  - every DMA lands in its own SBUF tile (no slot reuse -> DMAs carry no waits);
  - HWDGE queue assignment is round-robin over the combined SP+Act issue order;
    the interleave is chosen so every PE matmul group needs at most one new
    queue semaphore (xt shares q0 with W1 block 0; u1's q covers u3, u5's q
    covers u6);
  - 6-deep PSUM rotation (ht_ps[u%6]): the start=True overwrite of block u is
    ordered after layer-2 of block u-6, whose wait on the DVE evict semaphore
    already covers the reuse - no second wait;
  - DVE "touches" of the xt and w2 tiles acquire those DMA-queue semaphores on
    the DVE clock ahead of the evicts; layer-2 matmuls and the bias evicts
    inherit that coverage transitively and wait only on one semaphore;
  - the kernel-tail drain is split into single-wait drains (_patch_tail_drain).
Verified by _assert_wait_budget at build time.
"""

import numpy as np
import ml_dtypes

import concourse.bass as bass
import concourse.mybir as mybir
import concourse.tile_sem_assignment as _tsa
from concourse import tile
from concourse.bass_utils import run_bass_kernel_spmd

NUM_CAT = 8
B = 32
IN_DIM = 4096   # 16 * 256
MID = 1024
OUT = 512       # 16 * 32
P = 128
KT1 = IN_DIM // P    # 32 k-tiles for layer 1
KT2 = MID // P       # 8 mid-tiles (layer-1 out / layer-2 contraction)
NT = OUT // P        # 4 out-tiles
W1S = 64.0           # exact power-of-2 scale for fp8 W1
# Last W1 block's k-tiles split across the Act and Pool chains.
U7_SLABS = ((0, 16), (16, 16))
N_WARMUP = 6        # PE p-state warmup matmuls (bridge until first W1 block lands)
XB = KT1 * B         # 1024: xt columns inside the packed xt+bias tile
# packed tile columns: [0:1024] xt, [1024:1032] 64*b1T, [1032] zero,
# [1033:1161] b2 broadcast
XCOLS = XB + KT2 + 1 + NT * B
F32 = mybir.dt.float32
BF16 = mybir.dt.bfloat16
F8 = mybir.dt.float8e3  # e3m4

HWDGE_QUEUES = 4


class _PatchHwdgeQueues:
    """Pin Tile's HWDGE round-robin to n queues during scheduling."""

    def __init__(self, n: int):
        self.n = n

    def __enter__(self):
        self._saved = _tsa.NUM_HWDGE_SEMS
        _tsa.NUM_HWDGE_SEMS = self.n
        return self

    def __exit__(self, *exc):
        _tsa.NUM_HWDGE_SEMS = self._saved
        return False


def _patch_tail_drain():
    """Split Tile's kernel-tail drain (one wait per live proc) into a chain of
    single-wait drains: this walrus build caps sync-wait commands per instruction
    and rejects the stock multi-wait drain."""
    if getattr(tile.TileContext, "_tail_drain_patched", False):
        return
    from concourse.vector_clock import ScopedClock, VectorClock

    def _drain_and_barrier(self, tick_clock, wait_clock):
        gc = tick_clock.global_clock
        n = len(gc)
        for p in range(n):
            if gc[p] <= 0:
                continue
            sub = [0] * n
            sub[p] = gc[p]
            d = self.nc.sync.drain()
            wait_clock.add_sem_waits(d.ins, ScopedClock({None: VectorClock(sub)}))
        self.nc.all_engine_barrier()
        assert self.sems is not None
        popped = self.nc._tile_sem_poison_stack.pop()
        assert popped is self._sem_poison
        # No trailing all_engine_barrier: after the first barrier every engine
        # is quiescent, so the semaphore clear can be the program's last act -
        # re-execution still starts from zeroed semaphores.
        self.nc.clear_and_free_semaphores(list(self.sems.allocated().values()))

    tile.TileContext._drain_and_barrier = _drain_and_barrier
    tile.TileContext._tail_drain_patched = True


_patch_tail_drain()


def _build_nc() -> bass.Bass:
    nc = bass.Bass()

    # All DRAM layouts are pre-transposed on the host so each DMA partition
    # line is one contiguous segment.
    # Packed xt+bias tile (bf16): [p, 32t+b] = x_flat[b, 128t+p] for col<1024;
    # [p, 1024+u] = 64*b1[128u+p]; [p, 1032] = 0; [p, 1033+32v+b] = b2[128v+p]
    xt = nc.dram_tensor("xt", [P, XCOLS], BF16, kind="ExternalInput")
    # w1t[u, p, t, c] = 64 * W1[128t + p, 128u + c]   (fp8 e3m4)
    w1t = nc.dram_tensor("w1t", [KT2, P, KT1, P], F8, kind="ExternalInput")
    # w2t[p, u, n] = W2[128u + p, n] / 64
    w2t = nc.dram_tensor("w2t", [P, KT2, OUT], BF16, kind="ExternalInput")
    # out[p, v*32 + b] = out^T[128v + p, b]
    out = nc.dram_tensor("out", [P, NT * B], F32, kind="ExternalOutput")

    with _PatchHwdgeQueues(HWDGE_QUEUES), tile.TileContext(nc) as tc:
        with (
            tc.tile_pool(name="const", bufs=1) as const,
            tc.tile_pool(name="w1p", bufs=1) as w1p,
            tc.tile_pool(name="work", bufs=1) as work,
            tc.tile_pool(name="psum", bufs=1, space="PSUM") as psum,
        ):
            # --- DMA plan: three concurrent chains -------------------------
            #  SP  (HWDGE): w1u0, w1u3, w1u6, w1u7a(16kt)
            #  Act (HWDGE): xt+bias, w1u1, w1u5, w2b1(u4-5), w2b2(u6-7)
            #  Pool(SWDGE): w1u2, w1u4, w2a(u0-3), w1u7b(16kt), out
            # xt and u0 lead different chains, so the first matmul's data is
            # ready ~800 ns sooner than a serialized pair. HWDGE queue =
            # global (SP+Act) issue index mod 4; the interleave pins: u0 q0,
            # u3 q1, u6 q2, u7a q3, xt q0, u1 q1, u5 q2, w2b1 q3, w2b2 q0.
            # Single-wait coverage: L1u0 waits q0>=2 (u0+xt); L1u1 q1>=2
            # (covers u3); L1u5 q2>=2 (covers u6); L1u7a q3>=1. The w2 tiles
            # are covered on the DVE clock by touches; layer-2 matmuls inherit
            # that coverage through their single cumulative DVE wait.
            w1_sbs: dict = {}

            def w1_tile(u, k0, nk, name):
                t = w1p.tile([P, nk, P], F8, tag=f"w1_{name}", name=f"w1sb{name}")
                w1_sbs.setdefault(u, []).append((t, k0, nk))
                return t, w1t[u, :, k0 : k0 + nk, :]

            t, src = w1_tile(0, 0, KT1, "0")
            nc.sync.dma_start(t[:], src)                            # SP   q0
            t, src = w1_tile(3, 0, KT1, "3")
            nc.sync.dma_start(t[:], src)                            # SP   q1
            t, src = w1_tile(6, 0, KT1, "6")
            nc.sync.dma_start(t[:], src)                            # SP   q2
            k0, nk = U7_SLABS[0]
            t, src = w1_tile(7, k0, nk, "7a")
            nc.sync.dma_start(t[:], src)                            # SP   q3
            xt_sb = const.tile([P, XCOLS], BF16)
            nc.scalar.dma_start(xt_sb[:], xt[:])                    # Act  q0
            t, src = w1_tile(1, 0, KT1, "1")
            nc.scalar.dma_start(t[:], src)                          # Act  q1
            t, src = w1_tile(5, 0, KT1, "5")
            nc.scalar.dma_start(t[:], src)                          # Act  q2
            # w2 split: w2a rows u0-3 (Pool), w2b halves u4-5 / u6-7 (Act) so
            # each arrives before the layer-2 blocks that read it.
            w2a_sb = const.tile([P, 4, OUT], BF16, name="w2a")
            w2b1_sb = const.tile([P, 2, OUT], BF16, name="w2b1")
            w2b2_sb = const.tile([P, 2, OUT], BF16, name="w2b2")
            nc.scalar.dma_start(w2b1_sb[:], w2t[:, 4:6, :])         # Act  q3
            nc.scalar.dma_start(w2b2_sb[:], w2t[:, 6:8, :])         # Act  q0
            # Pool chain (SWDGE lanes, own semaphore space)
            for u in (2, 4):
                t, src = w1_tile(u, 0, KT1, str(u))
                nc.gpsimd.dma_start(t[:], src)
            nc.gpsimd.dma_start(w2a_sb[:], w2t[:, 0:4, :])
            k0, nk = U7_SLABS[1]
            t, src = w1_tile(7, k0, nk, "7b")
            nc.gpsimd.dma_start(t[:], src)
            # u7 sub-slabs must be consumed in issue order per engine; sort by k0
            w1_sbs[7].sort(key=lambda e: e[1])

            def w2_ap(u, v):
                if u < 4:
                    return w2a_sb[:, u, P * v : P * (v + 1)]
                if u < 6:
                    return w2b1_sb[:, u - 4, P * v : P * (v + 1)]
                return w2b2_sb[:, u - 6, P * v : P * (v + 1)]

            ht_ps = [
                psum.tile([P, B], F32, tag=f"hT_{i}", name=f"htps{i}")
                for i in range(6)
            ]
            ot_ps = psum.tile([P, NT * B], F32, tag="oT", name="otps")
            warm_ps = psum.tile([1, OUT], F32, tag="warm", name="warmps")
            ht_sb = work.tile([P, KT2, B], BF16)
            ot_sb = work.tile([P, NT * B], F32)

            zero_bc = xt_sb[:, XB + KT2 : XB + KT2 + 1].to_broadcast((P, B))

            # PE warmup: the tensor engine ramps to full clock only after 3us
            # of sustained work. A few dummy matmuls on a zeroed tile bridge
            # the wait for the first W1 block so the ramp starts at t~0.
            # N_WARMUP is tuned so the warmup ends right when block 0's data
            # lands - too few leaves a PE idle gap that resets the ramp.
            warm_sb = work.tile([1, OUT], BF16)
            nc.vector.memset(warm_sb[:], 0.0)
            for _ in range(N_WARMUP):
                nc.tensor.matmul(
                    warm_ps[:], warm_sb[0:1, 0:1], warm_sb[:], start=True, stop=True
                )

            def l1_block(u):
                ps = ht_ps[u % 6]
                for w1_sb, k0, nk in w1_sbs[u]:
                    for f in range(nk):
                        t = k0 + f
                        nc.tensor.matmul(
                            ps[:],
                            w1_sb[:, f, :],
                            xt_sb[:, B * t : B * (t + 1)],
                            start=(t == 0),
                            stop=(t == KT1 - 1),
                        )

            def evict_block(u):
                # ht = max(hT[u] + 64*b1T[:,u], 0), cast to bf16
                nc.vector.scalar_tensor_tensor(
                    ht_sb[:, u, :],
                    ht_ps[u % 6][:],
                    xt_sb[:, XB + u : XB + u + 1],
                    zero_bc,
                    mybir.AluOpType.add,
                    mybir.AluOpType.max,
                )

            def l2_block(u):
                # ot_ps is one PSUM bank = one zero region: a single
                # accumulation group for all 4 v slices. start on the very
                # first matmul zeroes the whole bank; stop on the very last.
                for v in range(NT):
                    nc.tensor.matmul(
                        ot_ps[:, B * v : B * (v + 1)],
                        w2_ap(u, v),
                        ht_sb[:, u, :],
                        start=(u == 0 and v == 0),
                        stop=(u == KT2 - 1 and v == NT - 1),
                    )

            # DVE touches: acquire the xt/w2 queue semaphores on the DVE
            # clock. A layer-2 matmul's single cumulative DVE wait is the max
            # position over its deps (its ht evict and its w2 touch), so the
            # w2 coverage reaches the PE transitively with no extra wait.
            # Touch placement in the DVE sequence matters only for stalls:
            # each touch sits after the evicts that would otherwise be queued
            # behind its later-arriving DMA.
            touch_sb = work.tile([P, 4], F32)
            nc.vector.tensor_copy(touch_sb[:, 0:1], xt_sb[:, XB : XB + 1])

            # Software pipeline. l2(0)/l2(1) precede l1(6)/l1(7) so the PE's
            # sync to the evict semaphore covers the ht_ps[u%6] reuse; the
            # remaining layer-2 blocks run at the end, overlapping e7.
            l1_block(0)
            evict_block(0)
            l1_block(1)
            evict_block(1)
            l1_block(2)
            evict_block(2)
            l1_block(3)
            evict_block(3)
            nc.vector.tensor_copy(touch_sb[:, 1:2], w2a_sb[:, 0, 0:1])
            nc.vector.tensor_copy(touch_sb[:, 2:3], w2b1_sb[:, 0, 0:1])
            l1_block(4)
            evict_block(4)
            l1_block(5)
            evict_block(5)
            l2_block(0)
            l1_block(6)
            evict_block(6)
            l2_block(1)
            nc.vector.tensor_copy(touch_sb[:, 3:4], w2b2_sb[:, 0, 0:1])
            l1_block(7)
            evict_block(7)
            for u in range(2, KT2):
                l2_block(u)

            # Final evict fuses the b2 add: ot = (oT + 0) + b2bc
            nc.vector.scalar_tensor_tensor(
                ot_sb[:],
                ot_ps[:],
                0.0,
                xt_sb[:, XB + KT2 + 1 : XCOLS],
                mybir.AluOpType.add,
                mybir.AluOpType.add,
            )
            # SWDGE store: a fresh DMA proc, so it carries only the DVE wait
            # (an HWDGE queue would add a FIFO wait -> 2 waits, and measures
            # slower besides).
            nc.gpsimd.dma_start(out[:], ot_sb[:])

    _assert_wait_budget(nc)
    return nc


def _assert_wait_budget(nc: bass.Bass, max_waits: int = 1):
    """This walrus build rejects instructions with >1 sync wait; fail fast."""
    bad = []
    for blk in nc.m.functions[0].blocks:
        for inst in blk.instructions:
            si = inst.sync_info
            nw = len(si.on_wait) if si is not None else 0
            if nw > max_waits:
                bad.append(
                    (
                        inst.name,
                        type(inst).__name__,
                        [(w.ant_name, w.wait_value) for w in si.on_wait],
                    )
                )
    if bad:
        raise RuntimeError(f"instructions with >{max_waits} sync waits: {bad}")


_NC_CACHE: bass.Bass | None = None


def _get_nc() -> bass.Bass:
    global _NC_CACHE
    if _NC_CACHE is None:
        _NC_CACHE = _build_nc()
    return _NC_CACHE


def _make_in_maps(x, W1, b1, W2, b2):
    bf16 = ml_dtypes.bfloat16
    f8 = ml_dtypes.float8_e3m4
    x_flat = np.asarray(x, dtype=np.float32).reshape(B, IN_DIM)
    W1 = np.asarray(W1, dtype=np.float32)
    W2 = np.asarray(W2, dtype=np.float32)
    b1 = np.asarray(b1, dtype=np.float32)
    b2 = np.asarray(b2, dtype=np.float32)
    # Packed xt+bias tile, per category (xt part shared): [p, 32t+b] = x^T;
    # then 64*b1T, a zero column, and b2 broadcast along the batch dim.
    xtb = np.zeros((NUM_CAT, P, XCOLS), np.float32)
    xtb[:, :, :XB] = (
        x_flat.reshape(B, KT1, P).transpose(2, 1, 0).reshape(1, P, XB)
    )
    xtb[:, :, XB : XB + KT2] = (b1 * W1S).reshape(NUM_CAT, KT2, P).transpose(0, 2, 1)
    b2t = b2.reshape(NUM_CAT, NT, P).transpose(0, 2, 1)  # [k, p, v]
    xtb[:, :, XB + KT2 + 1 :] = np.repeat(b2t, B, axis=2)
    xtb = np.ascontiguousarray(xtb.astype(bf16))
    # w1t[k][u, p, t, c] = 64 * W1[k, 128t + p, 128u + c], e3m4
    w1t = np.ascontiguousarray(
        (W1 * W1S).reshape(NUM_CAT, KT1, P, KT2, P).transpose(0, 3, 2, 1, 4)
    ).astype(f8)
    # w2t[k][p, u, n] = W2[k, 128u + p, n] / 64
    w2t = np.ascontiguousarray(
        (W2 / W1S).reshape(NUM_CAT, KT2, P, OUT).transpose(0, 2, 1, 3).astype(bf16)
    )
    return [
        {
            "xt": xtb[k],
            "w1t": np.ascontiguousarray(w1t[k]),
            "w2t": w2t[k],
        }
        for k in range(NUM_CAT)
    ]


def kernel(x, W1, b1, W2, b2, cat_ids) -> np.ndarray:
    nc = _get_nc()
    in_maps = _make_in_maps(x, W1, b1, W2, b2)
    res = run_bass_kernel_spmd(nc, in_maps, list(range(NUM_CAT))).results
    per_cat = np.stack(
        [np.asarray(res[k]["out"], dtype=np.float32) for k in range(NUM_CAT)]
    )  # [8, P, NT*B]
    cat = np.asarray(cat_ids).astype(np.int64).reshape(B)
    pc = per_cat.reshape(NUM_CAT, P, NT, B)
    sel = pc[cat, :, :, np.arange(B)]  # [B, P, NT]
    out = sel.transpose(0, 2, 1).reshape(B, OUT)  # o = 128v + p
    return np.ascontiguousarray(out.reshape(B, 16, 32).astype(np.float32))
